# revision 7
# baseline (speedup 1.0000x reference)
"""Contrastive loss kernel for Trainium2 (8 NeuronCores, data-parallel over B).

Reference math (B=16384, C=500, D=512):
    sq[b,c]  = |f_b|^2 + |p_c|^2 - 2 f_b.p_c
    d        = sqrt(max(sq, EPS))
    d_pos[b] = d[b, label[b]]
    d_neg[b] = min_{c != label[b]} d[b, c]
    loss     = mean(relu(d_pos - d_neg + 1))

Per-core plan (B_shard = 2048), v3:
  - Host supplies fp16 operands in matmul-ready layouts (layout+cast only):
    features_t16 [D, BS], prototypes2_t16 = 2*p.T [D, C], features natural
    fnat16 [128, NT*D] for the |f|^2 pass, labels as fp32 [128, NT].
  - Per b-tile, PSUM accumulates h_raw[b,c] = 2 f.p via 4 fp16 matmuls.
    No p2 / mask matmuls: gpsimd fuses the PSUM->SBUF fp16 copy with the
    (p2[c]-512) subtraction, so  h[b,c] = 2 f.p - p2[c] + 512.
  - DVE tensor_mask_reduce does the label handling in two passes, both
    windowed by the per-partition label index:
      M[b] = max_{c != lab}  h[b,c]   (wrap-inverted window excludes lab)
      Ppos[b] = h[b, lab]             (window [lab, lab+1))
    so d_neg^2 = f2 + 512 - M, d_pos^2 = f2 + 512 - Ppos.
  - f2 via one fused ACT Square+accumulate per natural-layout b-tile.
  - Tiny epilogue: sqrt (ACT, bias=512), relu(dpos - dneg + 1), row sum,
    ones-matmul partition sum -> scalar partial per core.
  - Host sums the 8 partials and divides by B.
"""

import numpy as np

import concourse.bacc as bacc
import concourse.bass as bass
import concourse.mybir as mybir
import concourse.tile as tile
from concourse import bass_utils

N_CORES = 8
B, C, D = 16384, 500, 512
BS = B // N_CORES            # 2048 rows per core
P = 128                      # partitions
NT = BS // P                 # 16 b-tiles per core
KD = D // P                  # 4 contraction tiles
NBC = BS // 512              # 4 b-chunks of 512 for ft DMA
MARGIN = 1.0
SHIFT = 512.0                # mean of p2 -- centers h for the fp16 pass
NEG_HUGE = -3.0e38
N_WARM = 14                  # PE warmup matmuls (p-state ramp) during DMA
F32 = mybir.dt.float32
F16 = mybir.dt.float16
AF = mybir.ActivationFunctionType
ALU = mybir.AluOpType


def _emit(tc):
    from contextlib import ExitStack

    ctx = ExitStack()
    with ctx:
        _emit_body(ctx, tc)


def _emit_body(ctx, tc):
    nc = tc.nc
    feat_t = nc.dram_tensor("features_t16", [D, BS], F16, kind="ExternalInput").ap()
    fnat = nc.dram_tensor("features_nat16", [P, NT * D], F16,
                          kind="ExternalInput").ap()
    prot2 = nc.dram_tensor("prototypes2_t16", [D, C], F16,
                           kind="ExternalInput").ap()
    labf = nc.dram_tensor("labels_f", [P, NT], F32, kind="ExternalInput").ap()
    out_dram = nc.dram_tensor("partial", [1, 1], F32, kind="ExternalOutput").ap()

    const_pool = ctx.enter_context(tc.tile_pool(name="const", bufs=1))
    big_pool = ctx.enter_context(tc.tile_pool(name="bigsb", bufs=1))
    h16_pool = ctx.enter_context(tc.tile_pool(name="h16", bufs=3))
    scr_pool = ctx.enter_context(tc.tile_pool(name="scr", bufs=2))
    sq_pool = ctx.enter_context(tc.tile_pool(name="sqf", bufs=2))
    acc_pool = ctx.enter_context(tc.tile_pool(name="acc", bufs=1))
    ps_h_pool = ctx.enter_context(tc.tile_pool(name="ps_h", bufs=3, space="PSUM"))
    ps_misc_pool = ctx.enter_context(tc.tile_pool(name="ps_misc", bufs=1,
                                                  space="PSUM"))

    # ---- constants -------------------------------------------------------
    ones1_16 = const_pool.tile([1, 1], F16)
    nc.vector.memset(ones1_16[:], 1.0)
    ones_row16 = const_pool.tile([1, P], F16)
    nc.vector.memset(ones_row16[:], 1.0)
    warm_row16 = const_pool.tile([1, 256], F16)
    nc.vector.memset(warm_row16[:], 0.0)
    quarter16 = const_pool.tile([P, 1], F16)
    nc.vector.memset(quarter16[:], 0.25)
    ones_col_f = const_pool.tile([P, 1], F32)
    nc.vector.memset(ones_col_f[:], 1.0)
    shift_col = const_pool.tile([P, 1], F32)
    nc.vector.memset(shift_col[:], SHIFT)

    labf_sb = const_pool.tile([P, NT], F32)
    nc.scalar.dma_start(labf_sb[:], labf[:])
    labf1_sb = const_pool.tile([P, NT], F32)
    nc.vector.tensor_scalar_add(labf1_sb[:], labf_sb[:], 1.0)

    # ---- PE warmup during the DMA load phase (HAM p-state ramp) ----------
    warm_ps = ps_misc_pool.tile([1, 256], F32, tag="warm")
    for _ in range(N_WARM):
        nc.tensor.matmul(warm_ps[:], ones1_16[:], warm_row16[:],
                         start=True, stop=True)

    # ---- big SBUF loads --------------------------------------------------
    # prototypes first (they gate every matmul), on their own queue
    pt2_sb = [big_pool.tile([P, C], F16, name=f"pt2_sb{k}") for k in range(KD)]
    for k in range(KD):
        nc.scalar.dma_start(pt2_sb[k][:], prot2[bass.ts(k, P), :])

    # fT k-tiles [128, 2048] fp16, DMA'd in 512-column chunks, bc-major and
    # spread over 3 DGE queues, so b-tiles 0-3 unblock early.
    ft_k = [big_pool.tile([P, BS], F16, name=f"ft_k{k}") for k in range(KD)]
    _qs = [nc.sync, nc.gpsimd, nc.scalar]
    for bc in range(NBC):
        for k in range(KD):
            _qs[(bc * KD + k) % 3].dma_start(
                ft_k[k][:, bass.ts(bc, 512)],
                feat_t[bass.ts(k, P), bass.ts(bc, 512)])

    # natural-layout features for the f2 pass (only needed as the main loop
    # reaches each tile, so they queue behind the ft chunks)
    fnat_sb = big_pool.tile([P, NT * D], F16, name="fnat")
    for t in range(NT):
        _qs[t % 3].dma_start(fnat_sb[:, bass.ts(t, D)],
                             fnat[:, bass.ts(t, D)])

    # ---- q_rep[b, c] = p2[c] - 512 (fp16, replicated on partitions) ------
    sq_pt = [big_pool.tile([P, C], F16, name=f"sq_pt{k}") for k in range(KD)]
    for k in range(KD):
        nc.scalar.activation(sq_pt[k][:], pt2_sb[k][:], AF.Square)
    p2row_ps = ps_misc_pool.tile([1, C], F32, tag="p2row")
    for k in range(KD):
        nc.tensor.matmul(p2row_ps[:], quarter16[:], sq_pt[k][:],
                         start=(k == 0), stop=(k == KD - 1))
    p2c16 = const_pool.tile([1, C], F16)
    nc.gpsimd.tensor_scalar_sub(p2c16[:], p2row_ps[:], SHIFT)
    q_ps = ps_misc_pool.tile([P, C], F32, tag="q_ps")
    nc.tensor.matmul(q_ps[:], ones_row16[:], p2c16[:], start=True, stop=True)
    q_rep = const_pool.tile([P, C], F16)
    nc.gpsimd.tensor_copy(q_rep[:], q_ps[:])

    # ---- accumulators ----------------------------------------------------
    m_all = acc_pool.tile([P, NT], F32)     # max_{c != lab} h
    p_all = acc_pool.tile([P, NT], F32)     # h[lab]
    f2_all = acc_pool.tile([P, NT], F32)    # |f|^2

    # ---- main loop over b-tiles -----------------------------------------
    for t in range(NT):
        h_ps = ps_h_pool.tile([P, C], F32)
        for k in range(KD):
            nc.tensor.matmul(h_ps[:], ft_k[k][:, bass.ts(t, P)],
                             pt2_sb[k][:], start=(k == 0), stop=(k == KD - 1))
        # h = 2 f.p - (p2 - 512), cast to fp16 in SBUF
        h16 = h16_pool.tile([P, C], F16, tag="h16")
        nc.gpsimd.tensor_sub(h16[:], h_ps[:], q_rep[:])

        # M = max_{c != lab} h   (wrapped window excludes the label column)
        scr_n = scr_pool.tile([P, C], F16, tag="scr_n")
        nc.vector.tensor_mask_reduce(
            scr_n[:], h16[:], labf1_sb[:, t:t + 1], labf_sb[:, t:t + 1],
            1.0, NEG_HUGE, ALU.max, accum_out=m_all[:, t:t + 1])
        # Ppos = h[lab]
        scr_p = scr_pool.tile([P, C], F16, tag="scr_p")
        nc.vector.tensor_mask_reduce(
            scr_p[:], h16[:], labf_sb[:, t:t + 1], labf1_sb[:, t:t + 1],
            1.0, NEG_HUGE, ALU.max, accum_out=p_all[:, t:t + 1])

        # f2 for this tile: fused square + row-sum on the scalar engine
        sq_f = sq_pool.tile([P, D], F16, tag="sq_f")
        nc.scalar.activation(sq_f[:], fnat_sb[:, bass.ts(t, D)], AF.Square,
                             accum_out=f2_all[:, t:t + 1])

    # ---- epilogue --------------------------------------------------------
    # d_neg^2 = f2 + 512 - M ; d_pos^2 = f2 + 512 - Ppos
    u = acc_pool.tile([P, NT], F32)
    nc.vector.tensor_sub(u[:], f2_all[:], m_all[:])
    dneg = acc_pool.tile([P, NT], F32)
    nc.scalar.activation(dneg[:], u[:], AF.Sqrt, bias=shift_col[:])
    v = acc_pool.tile([P, NT], F32)
    nc.vector.tensor_sub(v[:], f2_all[:], p_all[:])
    dpos = acc_pool.tile([P, NT], F32)
    nc.scalar.activation(dpos[:], v[:], AF.Sqrt, bias=shift_col[:])

    diff = acc_pool.tile([P, NT], F32)
    nc.vector.tensor_sub(diff[:], dpos[:], dneg[:])
    terms = acc_pool.tile([P, NT], F32)
    nc.vector.tensor_scalar(terms[:], diff[:], MARGIN, 0.0, ALU.add, ALU.max)
    row_sum = acc_pool.tile([P, 1], F32)
    nc.vector.tensor_reduce(row_sum[:], terms[:], mybir.AxisListType.X, ALU.add)

    tot_ps = ps_misc_pool.tile([1, 1], F32, tag="tot")
    nc.tensor.matmul(tot_ps[:], row_sum[:], ones_col_f[:],
                     start=True, stop=True)
    out_sb = acc_pool.tile([1, 1], F32)
    nc.vector.tensor_copy(out_sb[:], tot_ps[:])
    nc.sync.dma_start(out_dram[:], out_sb[:])


_NC_CACHE = None


def _get_nc():
    global _NC_CACHE
    if _NC_CACHE is None:
        nc = bacc.Bacc("TRN2", target_bir_lowering=False, debug=False,
                       num_devices=N_CORES)
        with tile.TileContext(nc) as tc:
            _emit(tc)
        nc.compile()
        _NC_CACHE = nc
    return _NC_CACHE


def _in_maps(features, prototypes, labels):
    features = np.asarray(features, dtype=np.float32)
    prototypes = np.asarray(prototypes, dtype=np.float32)
    labels = np.asarray(labels)
    prot2 = np.ascontiguousarray((2.0 * prototypes.T).astype(np.float16))  # [D, C]
    maps = []
    for i in range(N_CORES):
        fs = features[i * BS:(i + 1) * BS]                       # [BS, D]
        ft16 = np.ascontiguousarray(fs.T.astype(np.float16))     # [D, BS]
        # fnat16[p, t*D + d] = fs[t*128 + p, d]
        fnat16 = np.ascontiguousarray(
            fs.astype(np.float16).reshape(NT, P, D).transpose(1, 0, 2)
            .reshape(P, NT * D))
        ls = labels[i * BS:(i + 1) * BS].astype(np.float32)
        labf = np.ascontiguousarray(ls.reshape(NT, P).T)         # [P, NT]
        maps.append({
            "features_t16": ft16,
            "features_nat16": fnat16,
            "prototypes2_t16": prot2,
            "labels_f": labf,
        })
    return maps


def kernel(features, prototypes, labels, _trace=False):
    nc = _get_nc()
    maps = _in_maps(features, prototypes, labels)
    res = bass_utils.run_bass_kernel_spmd(
        nc, maps, core_ids=list(range(N_CORES)), trace=_trace)
    total = sum(float(r["partial"][0, 0]) for r in res.results)
    out = np.float32(total / B)
    if _trace:
        return out, res
    return out


# revision 8
# speedup vs baseline: 1.1132x; 1.1132x over previous
"""Contrastive loss kernel for Trainium2 (8 NeuronCores, data-parallel over B).

Reference math (B=16384, C=500, D=512):
    sq[b,c]  = |f_b|^2 + |p_c|^2 - 2 f_b.p_c
    d        = sqrt(max(sq, EPS))
    d_pos[b] = d[b, label[b]]
    d_neg[b] = min_{c != label[b]} d[b, c]
    loss     = mean(relu(d_pos - d_neg + 1))

Per-core plan (B_shard = 2048), v3:
  - Host supplies fp16 operands in matmul-ready layouts (layout+cast only):
    features_t16 [D, BS], prototypes2_t16 = 2*p.T [D, C], features natural
    fnat16 [128, NT*D] for the |f|^2 pass, labels as fp32 [128, NT].
  - Per b-tile, PSUM accumulates h_raw[b,c] = 2 f.p via 4 fp16 matmuls.
    No p2 / mask matmuls: gpsimd fuses the PSUM->SBUF fp16 copy with the
    (p2[c]-512) subtraction, so  h[b,c] = 2 f.p - p2[c] + 512.
  - DVE tensor_mask_reduce does the label handling in two passes, both
    windowed by the per-partition label index:
      M[b] = max_{c != lab}  h[b,c]   (wrap-inverted window excludes lab)
      Ppos[b] = h[b, lab]             (window [lab, lab+1))
    so d_neg^2 = f2 + 512 - M, d_pos^2 = f2 + 512 - Ppos.
  - f2 via one fused ACT Square+accumulate per natural-layout b-tile.
  - Tiny epilogue: sqrt (ACT, bias=512), relu(dpos - dneg + 1), row sum,
    ones-matmul partition sum -> scalar partial per core.
  - Host sums the 8 partials and divides by B.
"""

import numpy as np

import concourse.bacc as bacc
import concourse.bass as bass
import concourse.mybir as mybir
import concourse.tile as tile
from concourse import bass_utils

N_CORES = 8
B, C, D = 16384, 500, 512
BS = B // N_CORES            # 2048 rows per core
P = 128                      # partitions
NT = BS // P                 # 16 b-tiles per core
KD = D // P                  # 4 contraction tiles
NBC = BS // 512              # 4 b-chunks of 512 for ft DMA
MARGIN = 1.0
SHIFT = 512.0                # mean of p2 -- centers h for the fp16 pass
NEG_HUGE = -3.0e38
N_WARM = 12                  # PE warmup matmuls (p-state ramp) during DMA
F32 = mybir.dt.float32
F16 = mybir.dt.float16
F8 = mybir.dt.float8e4
AF = mybir.ActivationFunctionType
ALU = mybir.AluOpType


def _emit(tc):
    from contextlib import ExitStack

    ctx = ExitStack()
    with ctx:
        _emit_body(ctx, tc)


def _emit_body(ctx, tc):
    nc = tc.nc
    feat_t = nc.dram_tensor("features_t16", [D, BS], F16, kind="ExternalInput").ap()
    fnat = nc.dram_tensor("features_nat8", [P, NT * D], F8,
                          kind="ExternalInput").ap()
    prot2 = nc.dram_tensor("prototypes2_t16", [D, C], F16,
                           kind="ExternalInput").ap()
    labf = nc.dram_tensor("labels_f", [P, NT], F32, kind="ExternalInput").ap()
    out_dram = nc.dram_tensor("partial", [1, 1], F32, kind="ExternalOutput").ap()

    const_pool = ctx.enter_context(tc.tile_pool(name="const", bufs=1))
    big_pool = ctx.enter_context(tc.tile_pool(name="bigsb", bufs=1))
    h16_pool = ctx.enter_context(tc.tile_pool(name="h16", bufs=3))
    scr_pool = ctx.enter_context(tc.tile_pool(name="scr", bufs=2))
    sq_pool = ctx.enter_context(tc.tile_pool(name="sqf", bufs=2))
    acc_pool = ctx.enter_context(tc.tile_pool(name="acc", bufs=1))
    ps_h_pool = ctx.enter_context(tc.tile_pool(name="ps_h", bufs=3, space="PSUM"))
    ps_misc_pool = ctx.enter_context(tc.tile_pool(name="ps_misc", bufs=1,
                                                  space="PSUM"))

    # ---- constants -------------------------------------------------------
    ones1_16 = const_pool.tile([1, 1], F16)
    nc.vector.memset(ones1_16[:], 1.0)
    ones_row16 = const_pool.tile([1, P], F16)
    nc.vector.memset(ones_row16[:], 1.0)
    warm_row16 = const_pool.tile([1, 256], F16)
    nc.vector.memset(warm_row16[:], 0.0)
    quarter16 = const_pool.tile([P, 1], F16)
    nc.vector.memset(quarter16[:], 0.25)
    ones_col_f = const_pool.tile([P, 1], F32)
    nc.vector.memset(ones_col_f[:], 1.0)
    shift_col = const_pool.tile([P, 1], F32)
    nc.vector.memset(shift_col[:], SHIFT)

    labf_sb = const_pool.tile([P, NT], F32)
    nc.sync.dma_start(labf_sb[:], labf[:])
    labf1_sb = const_pool.tile([P, NT], F32)
    nc.vector.tensor_scalar_add(labf1_sb[:], labf_sb[:], 1.0)

    # ---- PE warmup during the DMA load phase (HAM p-state ramp) ----------
    warm_ps = ps_misc_pool.tile([1, 256], F32, tag="warm")
    for _ in range(N_WARM):
        nc.tensor.matmul(warm_ps[:], ones1_16[:], warm_row16[:],
                         start=True, stop=True)

    # ---- big SBUF loads --------------------------------------------------
    # prototypes first (they gate every matmul), on their own queue
    pt2_sb = [big_pool.tile([P, C], F16, name=f"pt2_sb{k}") for k in range(KD)]
    for k in range(KD):
        nc.sync.dma_start(pt2_sb[k][:], prot2[bass.ts(k, P), :])

    # fT k-tiles [128, 2048] fp16, DMA'd in 512-column chunks, bc-major and
    # spread over 3 DGE queues, so b-tiles 0-3 unblock early.
    ft_k = [big_pool.tile([P, BS], F16, name=f"ft_k{k}") for k in range(KD)]
    _ft_q = [nc.gpsimd, nc.scalar, nc.gpsimd, nc.sync,
             nc.gpsimd, nc.sync, nc.scalar, nc.gpsimd,
             nc.sync, nc.gpsimd, nc.sync, nc.sync,
             nc.scalar, nc.sync, nc.gpsimd, nc.sync]
    for bc in range(NBC):
        for k in range(KD):
            _ft_q[bc * KD + k].dma_start(
                ft_k[k][:, bass.ts(bc, 512)],
                feat_t[bass.ts(k, P), bass.ts(bc, 512)])

    # natural-layout features for the f2 pass (only needed as the main loop
    # reaches each tile, so they queue behind the ft chunks)
    fnat_sb = big_pool.tile([P, NT * D], F8, name="fnat")
    _fn_q = [nc.gpsimd, nc.sync, nc.gpsimd, nc.scalar] * 4
    for t in range(NT):
        _fn_q[t].dma_start(fnat_sb[:, bass.ts(t, D)],
                           fnat[:, bass.ts(t, D)])

    # ---- q_rep[b, c] = p2[c] - 512 (fp16, replicated on partitions) ------
    sq_pt = [big_pool.tile([P, C], F16, name=f"sq_pt{k}") for k in range(KD)]
    for k in range(KD):
        nc.gpsimd.tensor_mul(sq_pt[k][:], pt2_sb[k][:], pt2_sb[k][:])
    p2row_ps = ps_misc_pool.tile([1, C], F32, tag="p2row")
    for k in range(KD):
        nc.tensor.matmul(p2row_ps[:], quarter16[:], sq_pt[k][:],
                         start=(k == 0), stop=(k == KD - 1))
    p2c16 = const_pool.tile([1, C], F16)
    nc.gpsimd.tensor_scalar_sub(p2c16[:], p2row_ps[:], SHIFT)
    q_ps = ps_misc_pool.tile([P, C], F32, tag="q_ps")
    nc.tensor.matmul(q_ps[:], ones_row16[:], p2c16[:], start=True, stop=True)
    q_rep = const_pool.tile([P, C], F16)
    nc.gpsimd.tensor_copy(q_rep[:], q_ps[:])

    # ---- accumulators ----------------------------------------------------
    m_all = acc_pool.tile([P, NT], F32)     # max_{c != lab} h
    p_all = acc_pool.tile([P, NT], F32)     # h[lab]
    f2_all = acc_pool.tile([P, NT], F32)    # |f|^2

    # ---- main loop over b-tiles -----------------------------------------
    for t in range(NT):
        h_ps = ps_h_pool.tile([P, C], F32)
        for k in range(KD):
            nc.tensor.matmul(h_ps[:], ft_k[k][:, bass.ts(t, P)],
                             pt2_sb[k][:], start=(k == 0), stop=(k == KD - 1))
        # h = 2 f.p - (p2 - 512), cast to fp16 in SBUF
        h16 = h16_pool.tile([P, C], F16, tag="h16")
        nc.gpsimd.tensor_sub(h16[:], h_ps[:], q_rep[:])

        # M = max_{c != lab} h   (wrapped window excludes the label column)
        scr_n = scr_pool.tile([P, C], F16, tag="scr_n")
        nc.vector.tensor_mask_reduce(
            scr_n[:], h16[:], labf1_sb[:, t:t + 1], labf_sb[:, t:t + 1],
            1.0, NEG_HUGE, ALU.max, accum_out=m_all[:, t:t + 1])
        # Ppos = h[lab]
        scr_p = scr_pool.tile([P, C], F16, tag="scr_p")
        nc.vector.tensor_mask_reduce(
            scr_p[:], h16[:], labf_sb[:, t:t + 1], labf1_sb[:, t:t + 1],
            1.0, NEG_HUGE, ALU.max, accum_out=p_all[:, t:t + 1])

        # f2 for this tile: fused square + row-sum on the scalar engine
        sq_f = sq_pool.tile([P, D], F16, tag="sq_f")
        nc.scalar.activation(sq_f[:], fnat_sb[:, bass.ts(t, D)], AF.Square,
                             accum_out=f2_all[:, t:t + 1])

    # ---- epilogue --------------------------------------------------------
    # d_neg^2 = f2 + 512 - M ; d_pos^2 = f2 + 512 - Ppos
    u = acc_pool.tile([P, NT], F32)
    nc.vector.tensor_sub(u[:], f2_all[:], m_all[:])
    dneg = acc_pool.tile([P, NT], F32)
    nc.scalar.activation(dneg[:], u[:], AF.Sqrt, bias=shift_col[:])
    v = acc_pool.tile([P, NT], F32)
    nc.vector.tensor_sub(v[:], f2_all[:], p_all[:])
    dpos = acc_pool.tile([P, NT], F32)
    nc.scalar.activation(dpos[:], v[:], AF.Sqrt, bias=shift_col[:])

    diff = acc_pool.tile([P, NT], F32)
    nc.vector.tensor_sub(diff[:], dpos[:], dneg[:])
    terms = acc_pool.tile([P, NT], F32)
    nc.vector.tensor_scalar(terms[:], diff[:], MARGIN, 0.0, ALU.add, ALU.max)
    row_sum = acc_pool.tile([P, 1], F32)
    nc.vector.tensor_reduce(row_sum[:], terms[:], mybir.AxisListType.X, ALU.add)

    tot_ps = ps_misc_pool.tile([1, 1], F32, tag="tot")
    nc.tensor.matmul(tot_ps[:], row_sum[:], ones_col_f[:],
                     start=True, stop=True)
    out_sb = acc_pool.tile([1, 1], F32)
    nc.vector.tensor_copy(out_sb[:], tot_ps[:])
    nc.sync.dma_start(out_dram[:], out_sb[:])


_NC_CACHE = None


def _get_nc():
    global _NC_CACHE
    if _NC_CACHE is None:
        nc = bacc.Bacc("TRN2", target_bir_lowering=False, debug=False,
                       num_devices=N_CORES)
        with tile.TileContext(nc) as tc:
            _emit(tc)
        nc.compile()
        _NC_CACHE = nc
    return _NC_CACHE


def _in_maps(features, prototypes, labels):
    features = np.asarray(features, dtype=np.float32)
    prototypes = np.asarray(prototypes, dtype=np.float32)
    labels = np.asarray(labels)
    prot2 = np.ascontiguousarray((2.0 * prototypes.T).astype(np.float16))  # [D, C]
    maps = []
    for i in range(N_CORES):
        fs = features[i * BS:(i + 1) * BS]                       # [BS, D]
        ft16 = np.ascontiguousarray(fs.T.astype(np.float16))     # [D, BS]
        # fnat8[p, t*D + d] = fs[t*128 + p, d]
        fnat8 = np.ascontiguousarray(
            fs.reshape(NT, P, D).transpose(1, 0, 2)
            .reshape(P, NT * D).astype(mybir.dt.np(F8)))
        ls = labels[i * BS:(i + 1) * BS].astype(np.float32)
        labf = np.ascontiguousarray(ls.reshape(NT, P).T)         # [P, NT]
        maps.append({
            "features_t16": ft16,
            "features_nat8": fnat8,
            "prototypes2_t16": prot2,
            "labels_f": labf,
        })
    return maps


def kernel(features, prototypes, labels, _trace=False):
    nc = _get_nc()
    maps = _in_maps(features, prototypes, labels)
    res = bass_utils.run_bass_kernel_spmd(
        nc, maps, core_ids=list(range(N_CORES)), trace=_trace)
    total = sum(float(r["partial"][0, 0]) for r in res.results)
    out = np.float32(total / B)
    if _trace:
        return out, res
    return out
